# revision 25
# baseline (speedup 1.0000x reference)
"""Controlled-Rx gate on a 23-qubit state vector, Trainium2 Bass kernel.

State x (N=2^23 complex amplitudes) viewed as (control=2, target=2, rest),
control = qubit 0 (MSB), target = qubit 1.  The gate applies
M = [[c, -i s], [-i s, c]]  (c = cos(a/2), s = sin(a/2)) on the target
axis of the control=1 half; the control=0 half is untouched.

Real/imag parts (control=1 half):
    or0 = c*xr0 + s*xi1        oi0 = c*xi0 - s*xr1
    or1 = c*xr1 + s*xi0        oi1 = c*xi1 - s*xr0

Sharding: the rest axis is split evenly over 8 NeuronCores (pure data
parallel, no communication).  Each core streams 4 contiguous 1MB f32
input slices and writes 4 contiguous 1MB f32 output slices.  The
control=0 (identity) half never touches the device: it is copied during
the host-side complex64 assembly pass, which has to touch every output
element anyway.
"""

import math
import os

import numpy as np

import concourse.bass as bass
import concourse.mybir as mybir
from concourse.bass_utils import run_bass_kernel_spmd
from concourse.tile import TileContext

N = 8388608           # 2^23 amplitudes
R = N // 4            # rest axis size per (control, target) pair
NCORES = 8
RS = R // NCORES      # rest elements per core (262144)
P = 128               # SBUF partitions
CH = 2                # chunks per slice
FD = RS // (P * CH)   # free-dim columns per chunk tile

IN_NAMES = ("xr0", "xr1", "xi0", "xi1")
OUT_NAMES = ("or0", "oi0", "or1", "oi1")

# Stashed BassKernelResults from the last run (for test harness profiling).
_last_results = None
# Cached program (input-independent, reused across kernel() calls).
_nc_cache = None


def _legalize_waits(nc: bass.Bass) -> None:
    """This walrus build accepts only one sync-wait per instruction.  Tile's
    scheduler sometimes attaches 2+ (producer wait + DMA queue-head wait).
    Split the extras onto same-engine NoOp carriers placed immediately before
    the instruction: the engine sequencer stalls on those first, which is
    semantically identical."""
    for fn in nc.m.functions:
        for blk in fn.blocks:
            new_insts = []
            for inst in blk.instructions:
                si = inst.sync_info
                if si is not None and si.on_wait and len(si.on_wait) > 1:
                    extra, keep = si.on_wait[:-1], si.on_wait[-1:]
                    for w in extra:
                        new_insts.append(
                            mybir.InstNoOp(
                                name=nc.get_next_instruction_name(),
                                engine=inst.engine,
                                sync_info=mybir.SyncInfo(on_wait=[w], on_update=[]),
                                bass_nofuse=True,
                            )
                        )
                    si.on_wait = keep
                new_insts.append(inst)
            blk.instructions = new_insts


def _build_program(
    c: float = 0.0,
    s: float = 0.0,
    reps: int = 1,
    ch: int = CH,
    in_eng: str = "sync",
    out_eng: str = "scalar",
    bufs: int = 3,
    tmp_bufs: int = 2,
) -> bass.Bass:
    """reps>1 repeats the whole streaming body (same I/O regions, idempotent
    writes) so a benchmark can extract steady-state per-rep device time from
    wall-clock slopes."""
    nc = bass.Bass()
    f32 = mybir.dt.float32
    fd = RS // (P * ch)

    iv = {}
    for name in IN_NAMES:
        t = nc.dram_tensor(name, [RS], f32, kind="ExternalInput")
        iv[name] = t[:].rearrange("(k p f) -> k p f", p=P, f=fd)
    cs_in = nc.dram_tensor("cs", [P, 2], f32, kind="ExternalInput")
    ov = {}
    for name in OUT_NAMES:
        t = nc.dram_tensor(name, [RS], f32, kind="ExternalOutput")
        ov[name] = t[:].rearrange("(k p f) -> k p f", p=P, f=fd)

    # Benchmark mode (reps > 1): earlier reps write rotating scratch output
    # sets instead of the real outputs, so no tight WAW chain serializes the
    # steady-state stream; only the last rep writes the real outputs.  The
    # scratch sets are ExternalOutputs (not Internal) so walrus cannot DCE
    # them, and the reps-shaped dummy input makes the XLA module unique per
    # reps value (the BIR itself is not part of the jit cache key).
    NSCR = 2
    scr = []
    if reps > 1:
        nc.dram_tensor("bench_tag", [reps], f32, kind="ExternalInput")
    for q in range(min(NSCR, reps - 1)):
        scr.append({})
        for name in OUT_NAMES:
            t = nc.dram_tensor(f"scr{q}_{name}", [RS], f32, kind="ExternalOutput")
            scr[q][name] = t[:].rearrange("(k p f) -> k p f", p=P, f=fd)

    add = mybir.AluOpType.add
    sub = mybir.AluOpType.subtract
    mult = mybir.AluOpType.mult

    in_dma = getattr(nc, in_eng).dma_start
    out_dma = getattr(nc, out_eng).dma_start

    with TileContext(nc) as tc:
        with (
            tc.tile_pool(name="const", bufs=1) as const_pool,
            tc.tile_pool(name="io", bufs=bufs) as io_pool,
            tc.tile_pool(name="tmp", bufs=tmp_bufs) as tmp_pool,
        ):
            t_cs = const_pool.tile([P, 2], f32, name="t_cs")
            nc.gpsimd.dma_start(t_cs[:], cs_in[:])
            c_ap = t_cs[:, 0:1]
            s_ap = t_cs[:, 1:2]
            for j, k in [(j, k) for j in range(reps) for k in range(ch)]:
                dst = ov if j == reps - 1 else scr[j % NSCR]
                tin = {}
                for name in IN_NAMES:
                    tin[name] = io_pool.tile([P, fd], f32, name=f"in_{name}", tag=f"in_{name}")
                    in_dma(tin[name][:], iv[name][k])

                # t_<b> = s * <b> (vector engine, so the STT below depends on
                # it via same-engine program order, not a semaphore wait)
                ts = {}
                for name in IN_NAMES:
                    ts[name] = tmp_pool.tile([P, fd], f32, name=f"s_{name}", tag=f"s_{name}")
                    nc.vector.tensor_scalar_mul(ts[name][:], tin[name][:], s_ap)

                # out = (a * c) +/- t_b on the vector (DVE) engine
                for oname, a, b, op in (
                    ("or0", "xr0", "xi1", add),
                    ("oi0", "xi0", "xr1", sub),
                    ("or1", "xr1", "xi0", add),
                    ("oi1", "xi1", "xr0", sub),
                ):
                    to = io_pool.tile([P, fd], f32, name=f"out_{oname}", tag=f"out_{oname}")
                    nc.vector.scalar_tensor_tensor(
                        to[:], tin[a][:], c_ap, ts[b][:], mult, op
                    )
                    out_dma(dst[oname][k], to[:])
    _legalize_waits(nc)
    return nc


def _build_program_raw(ch: int = CH, ts_sync: bool = False,
                       detect_races: bool = True) -> bass.Bass:
    """Raw-Bass (no Tile) variant of the reps=1 streaming kernel.  Every tile
    is unique, so the only synchronization needed is: per-chunk load sems
    (all 4 loads of a chunk), one DVE progress sem gating each store, and a
    final store-completion wait.  Skips Tile's entry/exit barriers (~2.4us of
    a ~27us kernel).

    Layout per core: SP issues the 8 input loads, Pool loads the cs scalars,
    DVE computes (4x tensor_scalar + 4x scalar_tensor_tensor per chunk), ACT
    issues the 8 output stores on the second HWDGE ring."""
    import contextlib

    nc = bass.Bass(detect_race_conditions=detect_races)
    f32 = mybir.dt.float32
    fd = RS // (P * ch)

    iv = {}
    for name in IN_NAMES:
        t = nc.dram_tensor(name, [RS], f32, kind="ExternalInput")
        iv[name] = t[:].rearrange("(k p f) -> k p f", p=P, f=fd)
    cs_in = nc.dram_tensor("cs", [P, 2], f32, kind="ExternalInput")
    ov = {}
    for name in OUT_NAMES:
        t = nc.dram_tensor(name, [RS], f32, kind="ExternalOutput")
        ov[name] = t[:].rearrange("(k p f) -> k p f", p=P, f=fd)

    add = mybir.AluOpType.add
    sub = mybir.AluOpType.subtract
    mult = mybir.AluOpType.mult
    SPEC = (
        ("or0", "xr0", "xi1", add),
        ("oi0", "xi0", "xr1", sub),
        ("or1", "xr1", "xi0", add),
        ("oi1", "xi1", "xr0", sub),
    )

    with contextlib.ExitStack() as ctx:
        t_cs = ctx.enter_context(nc.sbuf_tensor("t_cs", [P, 2], f32))
        tin = {
            (name, k): ctx.enter_context(
                nc.sbuf_tensor(f"tin_{name}_{k}", [P, fd], f32)
            )
            for name in IN_NAMES
            for k in range(ch)
        }
        ttmp = {
            (o, k): ctx.enter_context(nc.sbuf_tensor(f"tt_{o}_{k}", [P, fd], f32))
            for o, _, _, _ in SPEC
            for k in range(ch)
        }
        tout = {
            (o, k): ctx.enter_context(nc.sbuf_tensor(f"to_{o}_{k}", [P, fd], f32))
            for o, _, _, _ in SPEC
            for k in range(ch)
        }
        cs_sem = ctx.enter_context(nc.semaphore("cs_sem"))
        ld_sems = [
            ctx.enter_context(nc.semaphore(f"ld_sem{k}")) for k in range(ch)
        ]
        cmp_sem = ctx.enter_context(nc.semaphore("cmp_sem"))
        ts_sem = ctx.enter_context(nc.semaphore("ts_sem"))
        st_sem = ctx.enter_context(nc.semaphore("st_sem"))
        block = ctx.enter_context(nc.Block())

        @block.gpsimd
        def _(gpsimd):
            gpsimd.dma_start(t_cs[:, :], cs_in[:]).then_inc(cs_sem, 16)

        @block.sync
        def _(sync):
            for k in range(ch):
                for name in IN_NAMES:
                    sync.dma_start(tin[name, k][:, :], iv[name][k]).then_inc(
                        ld_sems[k], 16
                    )

        @block.vector
        def _(vector):
            c_ap = t_cs[:, 0:1]
            s_ap = t_cs[:, 1:2]
            vector.wait_ge(cs_sem, 16)
            done = 0
            for k in range(ch):
                vector.wait_ge(ld_sems[k], 64)
                for o, a, b, op in SPEC:
                    # HW serializes consecutive DVE ops via the pipeline
                    # drain, so the same-engine TS->STT RAW is safe without a
                    # semaphore (Tile relies on this too).  ts_sync=True adds
                    # an explicit sem pair to satisfy CoreSim's race detector.
                    ts_i = nc.vector.tensor_scalar_mul(
                        ttmp[o, k][:, :], tin[b, k][:, :], s_ap
                    )
                    done += 1
                    if ts_sync:
                        ts_i.then_inc(ts_sem, 1)
                        vector.wait_ge(ts_sem, done)
                    nc.vector.scalar_tensor_tensor(
                        tout[o, k][:, :], tin[a, k][:, :], c_ap, ttmp[o, k][:, :],
                        mult, op,
                    ).then_inc(cmp_sem, 1)

        @block.scalar
        def _(scalar):
            t = 0
            for k in range(ch):
                for o, _, _, _ in SPEC:
                    t += 1
                    scalar.wait_ge(cmp_sem, t)
                    scalar.dma_start(ov[o][k], tout[o, k][:, :]).then_inc(st_sem, 16)
            scalar.wait_ge(st_sem, 16 * ch * 4)

    return nc


def kernel(x_real: np.ndarray, x_imag: np.ndarray, angle: np.ndarray) -> np.ndarray:
    global _last_results

    a = float(np.float64(np.asarray(angle).reshape(-1)[0]))
    c = float(np.float32(math.cos(0.5 * a)))
    s = float(np.float32(math.sin(0.5 * a)))

    xr = np.ascontiguousarray(x_real, dtype=np.float32).reshape(N)
    xi = np.ascontiguousarray(x_imag, dtype=np.float32).reshape(N)

    # The program is input-independent (angle arrives via the tiny cs input
    # tensor), so one build serves every call.
    global _nc_cache
    if _nc_cache is None:
        _nc_cache = _build_program()
    nc = _nc_cache
    cs = np.empty((P, 2), dtype=np.float32)
    cs[:, 0] = c
    cs[:, 1] = s

    in_maps = []
    for i in range(NCORES):
        lo0 = 2 * R + i * RS   # control=1, target=0
        lo1 = 3 * R + i * RS   # control=1, target=1
        in_maps.append(
            {
                "xr0": xr[lo0 : lo0 + RS],
                "xr1": xr[lo1 : lo1 + RS],
                "xi0": xi[lo0 : lo0 + RS],
                "xi1": xi[lo1 : lo1 + RS],
                "cs": cs,
            }
        )

    res = run_bass_kernel_spmd(
        nc,
        in_maps,
        list(range(NCORES)),
        trace=bool(os.environ.get("KERNEL_TRACE")),
    )
    _last_results = res

    out = np.empty((N,), dtype=np.complex64)
    # control=0 half: identity
    out.real[: 2 * R] = xr[: 2 * R]
    out.imag[: 2 * R] = xi[: 2 * R]
    for i in range(NCORES):
        r = res.results[i]
        lo0 = 2 * R + i * RS
        lo1 = 3 * R + i * RS
        out.real[lo0 : lo0 + RS] = r["or0"]
        out.imag[lo0 : lo0 + RS] = r["oi0"]
        out.real[lo1 : lo1 + RS] = r["or1"]
        out.imag[lo1 : lo1 + RS] = r["oi1"]
    return out.reshape(N, 1)
